# revision 9
# baseline (speedup 1.0000x reference)
"""Distributed (8-core) Trainium2 Bass kernel for nn_Attention.

Reference computation (per batch b of 4, x: [4, 256, 2048]):
  qkv = w_qkv @ x[b]            -> q,k,v each [8 heads, 64, 2048]
  dots = (q^T k) * 64**-0.5     -> [8, 2048, 2048]
  attn = softmax(dots, -1)
  av   = v @ attn^T             -> [8, 64, 2048]
  out  = w_out @ av + b_out     -> [256, 2048]

Sharding: 8 shards = (batch b in 0..3) x (query-half in 0..1). Each core
gets the full x[b] (columns permuted so its own 1024 query positions come
first), computes full k/v (duplicated with its half-partner, ~10% extra
flops, zero communication), q only for its 1024 queries, its half of the
attention, and its half of the final projection. Host concatenates.

On-core dataflow (f32 storage, float32r matmuls, fp32 softmax):
  q[hd,i], k[hd,j] natural; v computed TRANSPOSED [j, hd] via x-stationary
  matmuls. dots computed transposed [j-part, i-free] so AV contracts j on
  partitions. The AV stationary operand is [v_h | ones] ([128, 65]) so the
  softmax denominator accumulates as output partition 64. exp runs on
  ScalarE straight out of PSUM (scale folded), both heads of a pair in one
  [128, 1024] ACTIVATE.
"""

import sys

sys.path.insert(0, "/opt/trn_rl_repo")
sys.path.insert(0, "/root/.axon_site")

import numpy as np

DIM = 256
N = 2048
NQ = 1024
H = 8
DH = 64
HID = 512
PAIRS = 4
SCALE = DH ** -0.5

_CACHE = {}


def _register_ntff_hook():
    """The agent image's antenv lacks axon_hooks; synthesize it so
    run_bass_kernel_spmd(trace=True) can profile. Harmless if unused."""
    import types

    if "antenv.axon_hooks" in sys.modules:
        return
    try:
        import antenv
        from trn_agent_boot.trn_boot import _ntff_profile_via_ctypes

        mod = types.ModuleType("antenv.axon_hooks")
        _hook = [None]
        mod.set_axon_ntff_profile_hook = lambda h: _hook.__setitem__(0, h)
        mod.get_axon_ntff_profile_hook = lambda: _hook[0]
        sys.modules["antenv.axon_hooks"] = mod
        antenv.axon_hooks = mod
        mod.set_axon_ntff_profile_hook(
            _ntff_profile_via_ctypes("/opt/axon/libaxon_pjrt.so")
        )
    except Exception:
        pass


def build_nc():
    import concourse.mybir as mybir
    import concourse.tile as tile
    from concourse import bacc

    f32 = mybir.dt.float32
    bf16 = mybir.dt.bfloat16
    Exp = mybir.ActivationFunctionType.Exp

    nc = bacc.Bacc("TRN2", target_bir_lowering=False, debug=False)

    x_ext = nc.dram_tensor("x", [DIM, N], f32, kind="ExternalInput")
    wq_ext = nc.dram_tensor("wq_t", [DIM, HID], f32, kind="ExternalInput")
    wk_ext = nc.dram_tensor("wk_t", [DIM, HID], f32, kind="ExternalInput")
    wv_ext = nc.dram_tensor("wv_t", [DIM, HID], f32, kind="ExternalInput")
    wo_ext = nc.dram_tensor("wo_t", [HID, DIM], f32, kind="ExternalInput")
    b_ext = nc.dram_tensor("bias", [DIM, 1], f32, kind="ExternalInput")
    out_ext = nc.dram_tensor("out", [DIM, NQ], f32, kind="ExternalOutput")

    VSLOT = DH + 1  # 64 v columns + 1 ones column per head

    with tile.TileContext(nc) as tc:
        with (
            tc.tile_pool(name="persist", bufs=1) as pp,
            tc.tile_pool(name="stage", bufs=2) as stg,
            tc.tile_pool(name="qk", bufs=2) as qk,
            tc.tile_pool(name="epool", bufs=3) as ep,
            tc.tile_pool(name="small", bufs=2) as sp,
            tc.tile_pool(name="pdots", bufs=2, space="PSUM") as pd,
            tc.tile_pool(name="pattn", bufs=3, space="PSUM") as pa,
            tc.tile_pool(name="pproj", bufs=1, space="PSUM") as pj,
        ):
            # ---- warm the ACT exp table early (one tiny op) ----
            dummy = sp.tile([1, 1], f32, tag="dummy")
            nc.vector.memset(dummy[:], 0.0)
            dummy2 = sp.tile([1, 1], f32, tag="dummy2")
            nc.scalar.activation(dummy2[:], dummy[:], Exp)

            # ---- input DMAs (f32) + cast to bf16 ----
            def load_bf16(ext, rows, cols, tag):
                tiles = []
                for cc in range(rows // 128):
                    st = stg.tile([128, cols], f32, tag="stage", name="st")
                    nc.sync.dma_start(st[:], ext[cc * 128 : (cc + 1) * 128, :])
                    t = pp.tile([128, cols], bf16, tag=f"{tag}{cc}", name=f"{tag}{cc}")
                    nc.vector.tensor_copy(t[:], st[:])
                    tiles.append(t)
                return tiles

            x_sb = load_bf16(x_ext, DIM, N, "x")
            wq_sb = load_bf16(wq_ext, DIM, HID, "wq")
            wk_sb = load_bf16(wk_ext, DIM, HID, "wk")
            wv_sb = load_bf16(wv_ext, DIM, HID, "wv")
            wo_sb = load_bf16(wo_ext, HID, DIM, "wo")
            bias_sb = pp.tile([128, 2], f32, tag="bias")
            for oc in range(2):
                nc.sync.dma_start(
                    bias_sb[:, oc : oc + 1], b_ext[oc * 128 : (oc + 1) * 128, :]
                )

            # ---- v^T projection: vt[j, hd] for all heads, x chunks stationary ----
            # vt layout per j-chunk: 8 slots of [64 v | 1 ones]
            def qk_proj(p):
                q_t = qk.tile([128, NQ], bf16, tag="q", name="q_t")
                for ic in range(2):
                    ps = pj.tile([128, 512], f32, tag="proj", name="ps")
                    for cc in range(2):
                        nc.tensor.matmul(
                            ps[:],
                            lhsT=wq_sb[cc][:, p * 128 : (p + 1) * 128],
                            rhs=x_sb[cc][:, ic * 512 : (ic + 1) * 512],
                            start=(cc == 0),
                            stop=(cc == 1),
                        )
                    nc.vector.tensor_copy(q_t[:, ic * 512 : (ic + 1) * 512], ps[:])
                k_t = qk.tile([128, N], bf16, tag="k", name="k_t")
                for jc4 in range(4):
                    ps = pj.tile([128, 512], f32, tag="proj", name="ps")
                    for cc in range(2):
                        nc.tensor.matmul(
                            ps[:],
                            lhsT=wk_sb[cc][:, p * 128 : (p + 1) * 128],
                            rhs=x_sb[cc][:, jc4 * 512 : (jc4 + 1) * 512],
                            start=(cc == 0),
                            stop=(cc == 1),
                        )
                    nc.vector.tensor_copy(k_t[:, jc4 * 512 : (jc4 + 1) * 512], ps[:])
                return q_t, k_t

            qk0 = qk_proj(0)

            ones_sb = pp.tile([128, H], f32, tag="ones")
            nc.vector.memset(ones_sb[:], 1.0)
            vt = pp.tile([128, 16 * H * VSLOT], bf16, tag="vt")
            for jc in range(16):
                ps = pj.tile([128, HID], f32, tag="proj", name="ps")
                for cc in range(2):
                    nc.tensor.matmul(
                        ps[:],
                        lhsT=x_sb[cc][:, jc * 128 : (jc + 1) * 128],
                        rhs=wv_sb[cc][:],
                        start=(cc == 0),
                        stop=(cc == 1),
                    )
                vslice = vt[
                    :, jc * H * VSLOT : (jc + 1) * H * VSLOT
                ].rearrange("p (h s) -> p h s", s=VSLOT)
                nc.vector.tensor_copy(
                    vslice[:, :, 0:DH],
                    ps[:].rearrange("p (h d) -> p h d", d=DH),
                )
                nc.vector.tensor_copy(
                    vslice[:, :, DH : DH + 1],
                    ones_sb[:].rearrange("p (h o) -> p h o", o=1),
                )

            attn_n = [
                pp.tile([128, NQ], bf16, tag=f"attn_n{p}", name=f"attn_n{p}")
                for p in range(PAIRS)
            ]

            # ---- per head-pair: q/k projection then attention ----
            out_acc = [
                pp.tile([128, NQ], f32, tag=f"oacc{oc}", name=f"oacc{oc}")
                for oc in range(2)
            ]
            for p in range(PAIRS):
                q_t, k_t = qk0 if p == 0 else qk_proj(p)

                for ic in range(2):
                    attA = pa.tile([128, 512], f32, tag="att", name="attA")
                    attB = pa.tile([128, 512], f32, tag="att", name="attB")
                    for jc in range(16):
                        d = pd.tile([128, 1024], f32, tag="dots", name="d")
                        # head A = 2p (k rows 0:64), head B = 2p+1 (rows 64:128)
                        nc.tensor.matmul(
                            d[:, 0:512],
                            lhsT=k_t[0:64, jc * 128 : (jc + 1) * 128],
                            rhs=q_t[0:64, ic * 512 : (ic + 1) * 512],
                            start=True,
                            stop=True,
                        )
                        nc.tensor.matmul(
                            d[:, 512:1024],
                            lhsT=k_t[64:128, jc * 128 : (jc + 1) * 128],
                            rhs=q_t[64:128, ic * 512 : (ic + 1) * 512],
                            start=True,
                            stop=True,
                        )
                        e = ep.tile([128, 1024], bf16, tag="e")
                        nc.scalar.activation(e[:], d[:], Exp, scale=SCALE)
                        base = jc * H * VSLOT
                        hA = 2 * p
                        hB = 2 * p + 1
                        nc.tensor.matmul(
                            attA[0 : DH + 1, :],
                            lhsT=vt[:, base + hA * VSLOT : base + (hA + 1) * VSLOT],
                            rhs=e[:, 0:512],
                            start=(jc == 0),
                            stop=(jc == 15),
                        )
                        nc.tensor.matmul(
                            attB[0 : DH + 1, :],
                            lhsT=vt[:, base + hB * VSLOT : base + (hB + 1) * VSLOT],
                            rhs=e[:, 512:1024],
                            start=(jc == 0),
                            stop=(jc == 15),
                        )
                    # normalize: rows 0:64 / row 64, into attn_n[p]
                    for hh, att in ((0, attA), (1, attB)):
                        den = sp.tile([1, 512], f32, tag="den")
                        nc.vector.tensor_copy(den[:], att[64:65, :])
                        rec = sp.tile([1, 512], f32, tag="rec")
                        nc.vector.reciprocal_approx_fast(rec[:], den[:])
                        recb = sp.tile([64, 512], f32, tag="recb")
                        nc.gpsimd.partition_broadcast(recb[:], rec[:])
                        nc.vector.tensor_mul(
                            attn_n[p][hh * 64 : (hh + 1) * 64, ic * 512 : (ic + 1) * 512],
                            att[0:64, :],
                            recb[:],
                        )

                # partial output projection for this pair, accumulated in SBUF
                for oc in range(2):
                    for ic2 in range(2):
                        ps = pj.tile([128, 512], f32, tag="proj", name="ops")
                        nc.tensor.matmul(
                            ps[:],
                            lhsT=wo_sb[p][:, oc * 128 : (oc + 1) * 128],
                            rhs=attn_n[p][:, ic2 * 512 : (ic2 + 1) * 512],
                            start=True,
                            stop=True,
                        )
                        dst = out_acc[oc][:, ic2 * 512 : (ic2 + 1) * 512]
                        if p == 0:
                            nc.vector.tensor_scalar_add(
                                dst, ps[:], bias_sb[:, oc : oc + 1]
                            )
                        else:
                            nc.vector.tensor_add(dst, dst, ps[:])

            for oc in range(2):
                nc.sync.dma_start(out_ext[oc * 128 : (oc + 1) * 128, :], out_acc[oc][:])

    nc.compile()
    return nc


def _shard_inputs(x, w_qkv, w_out, b_out):
    """Returns in_maps for cores 0..7; core c = (batch c//2, query-half c%2)."""
    x = np.asarray(x, dtype=np.float32)
    w_qkv = np.asarray(w_qkv, dtype=np.float32)
    w_out = np.asarray(w_out, dtype=np.float32)
    b_out = np.asarray(b_out, dtype=np.float32)

    wq_t = np.ascontiguousarray(w_qkv[0:HID].T)  # [256, 512]
    wk_t = np.ascontiguousarray(w_qkv[HID : 2 * HID].T)
    wv_t = np.ascontiguousarray(w_qkv[2 * HID : 3 * HID].T)
    wo_t = np.ascontiguousarray(w_out.T)  # [512, 256]
    bias = np.ascontiguousarray(b_out.reshape(DIM, 1))

    in_maps = []
    for c in range(8):
        b, half = divmod(c, 2)
        xb = x[b]
        halves = [xb[:, 0:NQ], xb[:, NQ:N]]
        x_perm = np.ascontiguousarray(
            np.concatenate([halves[half], halves[1 - half]], axis=1)
        )
        in_maps.append(
            {
                "x": x_perm,
                "wq_t": wq_t,
                "wk_t": wk_t,
                "wv_t": wv_t,
                "wo_t": wo_t,
                "bias": bias,
            }
        )
    return in_maps


def run(x, w_qkv, w_out, b_out, trace=False, tmpdir=None):
    from concourse.bass_utils import run_bass_kernel_spmd

    _register_ntff_hook()
    if "nc" not in _CACHE:
        _CACHE["nc"] = build_nc()
    nc = _CACHE["nc"]
    in_maps = _shard_inputs(x, w_qkv, w_out, b_out)
    kw = {}
    if trace:
        kw.update(trace=True, tmpdir=tmpdir)
    res = run_bass_kernel_spmd(nc, in_maps, core_ids=list(range(8)), **kw)
    out = np.empty((4, DIM, N), dtype=np.float32)
    for c in range(8):
        b, half = divmod(c, 2)
        out[b][:, half * NQ : (half + 1) * NQ] = res.results[c]["out"]
    return out, res


def kernel(**inputs):
    out, _ = run(
        inputs["x"], inputs["w_qkv"], inputs["w_out"], inputs["b_out"]
    )
    return out


# revision 10
# speedup vs baseline: 1.2276x; 1.2276x over previous
"""Distributed (8-core) Trainium2 Bass kernel for nn_Attention.

Reference computation (per batch b of 4, x: [4, 256, 2048]):
  qkv = w_qkv @ x[b]            -> q,k,v each [8 heads, 64, 2048]
  dots = (q^T k) * 64**-0.5     -> [8, 2048, 2048]
  attn = softmax(dots, -1)
  av   = v @ attn^T             -> [8, 64, 2048]
  out  = w_out @ av + b_out     -> [256, 2048]

Sharding: 8 shards = (batch b in 0..3) x (query-half in 0..1). Each core
gets the full x[b] (columns permuted so its own 1024 query positions come
first), computes full k/v (duplicated with its half-partner, ~10% extra
flops, zero communication), q only for its 1024 queries, its half of the
attention, and its half of the final projection. Host concatenates.

On-core dataflow (f32 storage, float32r matmuls, fp32 softmax):
  q[hd,i], k[hd,j] natural; v computed TRANSPOSED [j, hd] via x-stationary
  matmuls. dots computed transposed [j-part, i-free] so AV contracts j on
  partitions. The AV stationary operand is [v_h | ones] ([128, 65]) so the
  softmax denominator accumulates as output partition 64. exp runs on
  ScalarE straight out of PSUM (scale folded), both heads of a pair in one
  [128, 1024] ACTIVATE.
"""

import sys

sys.path.insert(0, "/opt/trn_rl_repo")
sys.path.insert(0, "/root/.axon_site")

import numpy as np

DIM = 256
N = 2048
NQ = 1024
H = 8
DH = 64
HID = 512
PAIRS = 4
SCALE = DH ** -0.5

_CACHE = {}


def _register_ntff_hook():
    """The agent image's antenv lacks axon_hooks; synthesize it so
    run_bass_kernel_spmd(trace=True) can profile. Harmless if unused."""
    import types

    if "antenv.axon_hooks" in sys.modules:
        return
    try:
        import antenv
        from trn_agent_boot.trn_boot import _ntff_profile_via_ctypes

        mod = types.ModuleType("antenv.axon_hooks")
        _hook = [None]
        mod.set_axon_ntff_profile_hook = lambda h: _hook.__setitem__(0, h)
        mod.get_axon_ntff_profile_hook = lambda: _hook[0]
        sys.modules["antenv.axon_hooks"] = mod
        antenv.axon_hooks = mod
        mod.set_axon_ntff_profile_hook(
            _ntff_profile_via_ctypes("/opt/axon/libaxon_pjrt.so")
        )
    except Exception:
        pass


def build_nc():
    import concourse.mybir as mybir
    import concourse.tile as tile
    from concourse import bacc

    f32 = mybir.dt.float32
    bf16 = mybir.dt.bfloat16
    Exp = mybir.ActivationFunctionType.Exp

    nc = bacc.Bacc("TRN2", target_bir_lowering=False, debug=False)

    x_ext = nc.dram_tensor("x", [DIM, N], f32, kind="ExternalInput")
    wq_ext = nc.dram_tensor("wq_t", [DIM, HID], f32, kind="ExternalInput")
    wk_ext = nc.dram_tensor("wk_t", [DIM, HID], f32, kind="ExternalInput")
    wv_ext = nc.dram_tensor("wv_t", [DIM, HID], f32, kind="ExternalInput")
    wo_ext = nc.dram_tensor("wo_t", [HID, DIM], f32, kind="ExternalInput")
    b_ext = nc.dram_tensor("bias", [DIM, 1], f32, kind="ExternalInput")
    out_ext = nc.dram_tensor("out", [DIM, NQ], f32, kind="ExternalOutput")

    VSLOT = DH + 1  # 64 v columns + 1 ones column per head

    with tile.TileContext(nc) as tc:
        with (
            tc.tile_pool(name="persist", bufs=1) as pp,
            tc.tile_pool(name="stage", bufs=2) as stg,
            tc.tile_pool(name="qk", bufs=2) as qk,
            tc.tile_pool(name="epool", bufs=3) as ep,
            tc.tile_pool(name="small", bufs=2) as sp,
            tc.tile_pool(name="pdots", bufs=2, space="PSUM") as pd,
            tc.tile_pool(name="pattn", bufs=3, space="PSUM") as pa,
            tc.tile_pool(name="pproj", bufs=1, space="PSUM") as pj,
        ):
            # ---- warm the ACT exp table early (one tiny op) ----
            dummy = sp.tile([1, 1], f32, tag="dummy")
            nc.vector.memset(dummy[:], 0.0)
            dummy2 = sp.tile([1, 1], f32, tag="dummy2")
            nc.scalar.activation(dummy2[:], dummy[:], Exp)

            # ---- input DMAs (f32) + cast to bf16 ----
            def load_bf16(ext, rows, cols, tag):
                tiles = []
                for cc in range(rows // 128):
                    st = stg.tile([128, cols], f32, tag="stage", name="st")
                    nc.sync.dma_start(st[:], ext[cc * 128 : (cc + 1) * 128, :])
                    t = pp.tile([128, cols], bf16, tag=f"{tag}{cc}", name=f"{tag}{cc}")
                    nc.vector.tensor_copy(t[:], st[:])
                    tiles.append(t)
                return tiles

            x_sb = load_bf16(x_ext, DIM, N, "x")
            wq_sb = load_bf16(wq_ext, DIM, HID, "wq")
            wk_sb = load_bf16(wk_ext, DIM, HID, "wk")
            wv_sb = load_bf16(wv_ext, DIM, HID, "wv")
            wo_sb = load_bf16(wo_ext, HID, DIM, "wo")
            bias_sb = pp.tile([128, 2], f32, tag="bias")
            for oc in range(2):
                nc.sync.dma_start(
                    bias_sb[:, oc : oc + 1], b_ext[oc * 128 : (oc + 1) * 128, :]
                )

            # ---- v^T projection: vt[j, hd] for all heads, x chunks stationary ----
            # vt layout per j-chunk: 8 slots of [64 v | 1 ones]
            def qk_proj(p):
                q_t = qk.tile([128, NQ], bf16, tag="q", name="q_t")
                for ic in range(2):
                    ps = pj.tile([128, 512], f32, tag="proj", name="ps")
                    for cc in range(2):
                        nc.tensor.matmul(
                            ps[:],
                            lhsT=wq_sb[cc][:, p * 128 : (p + 1) * 128],
                            rhs=x_sb[cc][:, ic * 512 : (ic + 1) * 512],
                            start=(cc == 0),
                            stop=(cc == 1),
                        )
                    nc.vector.tensor_copy(q_t[:, ic * 512 : (ic + 1) * 512], ps[:])
                k_t = qk.tile([128, N], bf16, tag="k", name="k_t")
                for jc4 in range(4):
                    ps = pj.tile([128, 512], f32, tag="proj", name="ps")
                    for cc in range(2):
                        nc.tensor.matmul(
                            ps[:],
                            lhsT=wk_sb[cc][:, p * 128 : (p + 1) * 128],
                            rhs=x_sb[cc][:, jc4 * 512 : (jc4 + 1) * 512],
                            start=(cc == 0),
                            stop=(cc == 1),
                        )
                    nc.vector.tensor_copy(k_t[:, jc4 * 512 : (jc4 + 1) * 512], ps[:])
                return q_t, k_t

            qk0 = qk_proj(0)

            ones_sb = pp.tile([128, H], f32, tag="ones")
            nc.vector.memset(ones_sb[:], 1.0)
            vt = pp.tile([128, 16 * H * VSLOT], bf16, tag="vt")
            for jc in range(16):
                ps = pj.tile([128, HID], f32, tag="proj", name="ps")
                for cc in range(2):
                    nc.tensor.matmul(
                        ps[:],
                        lhsT=x_sb[cc][:, jc * 128 : (jc + 1) * 128],
                        rhs=wv_sb[cc][:],
                        start=(cc == 0),
                        stop=(cc == 1),
                    )
                vslice = vt[
                    :, jc * H * VSLOT : (jc + 1) * H * VSLOT
                ].rearrange("p (h s) -> p h s", s=VSLOT)
                nc.vector.tensor_copy(
                    vslice[:, :, 0:DH],
                    ps[:].rearrange("p (h d) -> p h d", d=DH),
                )
                nc.vector.tensor_copy(
                    vslice[:, :, DH : DH + 1],
                    ones_sb[:].rearrange("p (h o) -> p h o", o=1),
                )

            attn_n = [
                pp.tile([128, NQ], bf16, tag=f"attn_n{p}", name=f"attn_n{p}")
                for p in range(PAIRS)
            ]

            # ---- per head-pair: q/k projection then attention ----
            out_acc = [
                pp.tile([128, NQ], f32, tag=f"oacc{oc}", name=f"oacc{oc}")
                for oc in range(2)
            ]
            qk_next = qk0
            for p in range(PAIRS):
                q_t, k_t = qk_next

                for ic in range(2):
                    if ic == 1 and p < PAIRS - 1:
                        # emit next pair's projections here so their PSUM-pool
                        # slots precede this pair's out-proj tiles (lets the
                        # scheduler hoist them into this pair's attention)
                        qk_next = qk_proj(p + 1)
                    attA = pa.tile([128, 512], f32, tag="att", name="attA")
                    attB = pa.tile([128, 512], f32, tag="att", name="attB")
                    for jc in range(16):
                        d = pd.tile([128, 1024], f32, tag="dots", name="d")
                        # head A = 2p (k rows 0:64), head B = 2p+1 (rows 64:128)
                        nc.tensor.matmul(
                            d[:, 0:512],
                            lhsT=k_t[0:64, jc * 128 : (jc + 1) * 128],
                            rhs=q_t[0:64, ic * 512 : (ic + 1) * 512],
                            start=True,
                            stop=True,
                        )
                        nc.tensor.matmul(
                            d[:, 512:1024],
                            lhsT=k_t[64:128, jc * 128 : (jc + 1) * 128],
                            rhs=q_t[64:128, ic * 512 : (ic + 1) * 512],
                            start=True,
                            stop=True,
                        )
                        e = ep.tile([128, 1024], bf16, tag="e")
                        nc.scalar.activation(e[:], d[:], Exp, scale=SCALE)
                        base = jc * H * VSLOT
                        hA = 2 * p
                        hB = 2 * p + 1
                        nc.tensor.matmul(
                            attA[0 : DH + 1, :],
                            lhsT=vt[:, base + hA * VSLOT : base + (hA + 1) * VSLOT],
                            rhs=e[:, 0:512],
                            start=(jc == 0),
                            stop=(jc == 15),
                        )
                        nc.tensor.matmul(
                            attB[0 : DH + 1, :],
                            lhsT=vt[:, base + hB * VSLOT : base + (hB + 1) * VSLOT],
                            rhs=e[:, 512:1024],
                            start=(jc == 0),
                            stop=(jc == 15),
                        )
                    # normalize: rows 0:64 / row 64, into attn_n[p]
                    for hh, att in ((0, attA), (1, attB)):
                        den = sp.tile([1, 512], f32, tag="den")
                        nc.vector.tensor_copy(den[:], att[64:65, :])
                        rec = sp.tile([1, 512], f32, tag="rec")
                        nc.vector.reciprocal_approx_fast(rec[:], den[:])
                        recb = sp.tile([64, 512], f32, tag="recb")
                        nc.gpsimd.partition_broadcast(recb[:], rec[:])
                        nc.vector.tensor_mul(
                            attn_n[p][hh * 64 : (hh + 1) * 64, ic * 512 : (ic + 1) * 512],
                            att[0:64, :],
                            recb[:],
                        )

                # partial output projection for this pair, accumulated in SBUF
                for oc in range(2):
                    for ic2 in range(2):
                        ps = pj.tile([128, 512], f32, tag="proj", name="ops")
                        nc.tensor.matmul(
                            ps[:],
                            lhsT=wo_sb[p][:, oc * 128 : (oc + 1) * 128],
                            rhs=attn_n[p][:, ic2 * 512 : (ic2 + 1) * 512],
                            start=True,
                            stop=True,
                        )
                        dst = out_acc[oc][:, ic2 * 512 : (ic2 + 1) * 512]
                        if p == 0:
                            nc.vector.tensor_scalar_add(
                                dst, ps[:], bias_sb[:, oc : oc + 1]
                            )
                        else:
                            nc.vector.tensor_add(dst, dst, ps[:])

            for oc in range(2):
                nc.sync.dma_start(out_ext[oc * 128 : (oc + 1) * 128, :], out_acc[oc][:])

    nc.compile()
    return nc


def _shard_inputs(x, w_qkv, w_out, b_out):
    """Returns in_maps for cores 0..7; core c = (batch c//2, query-half c%2)."""
    x = np.asarray(x, dtype=np.float32)
    w_qkv = np.asarray(w_qkv, dtype=np.float32)
    w_out = np.asarray(w_out, dtype=np.float32)
    b_out = np.asarray(b_out, dtype=np.float32)

    wq_t = np.ascontiguousarray(w_qkv[0:HID].T)  # [256, 512]
    wk_t = np.ascontiguousarray(w_qkv[HID : 2 * HID].T)
    wv_t = np.ascontiguousarray(w_qkv[2 * HID : 3 * HID].T)
    wo_t = np.ascontiguousarray(w_out.T)  # [512, 256]
    bias = np.ascontiguousarray(b_out.reshape(DIM, 1))

    in_maps = []
    for c in range(8):
        b, half = divmod(c, 2)
        xb = x[b]
        halves = [xb[:, 0:NQ], xb[:, NQ:N]]
        x_perm = np.ascontiguousarray(
            np.concatenate([halves[half], halves[1 - half]], axis=1)
        )
        in_maps.append(
            {
                "x": x_perm,
                "wq_t": wq_t,
                "wk_t": wk_t,
                "wv_t": wv_t,
                "wo_t": wo_t,
                "bias": bias,
            }
        )
    return in_maps


def run(x, w_qkv, w_out, b_out, trace=False, tmpdir=None):
    from concourse.bass_utils import run_bass_kernel_spmd

    _register_ntff_hook()
    if "nc" not in _CACHE:
        _CACHE["nc"] = build_nc()
    nc = _CACHE["nc"]
    in_maps = _shard_inputs(x, w_qkv, w_out, b_out)
    kw = {}
    if trace:
        kw.update(trace=True, tmpdir=tmpdir)
    res = run_bass_kernel_spmd(nc, in_maps, core_ids=list(range(8)), **kw)
    out = np.empty((4, DIM, N), dtype=np.float32)
    for c in range(8):
        b, half = divmod(c, 2)
        out[b][:, half * NQ : (half + 1) * NQ] = res.results[c]["out"]
    return out, res


def kernel(**inputs):
    out, _ = run(
        inputs["x"], inputs["w_qkv"], inputs["w_out"], inputs["b_out"]
    )
    return out
